# revision 30
# baseline (speedup 1.0000x reference)
"""Trainium2 Bass kernel for ConvTranspose3d(3->16,k3,s2,p1) + BatchNorm3d(train) + 2x AvgPool3d(2).

v2 design (bf16, multi-ring DMA, quad-batched scans):
  - All conv inputs/weights in bf16 (tolerance 2e-2 >> bf16 noise ~1e-3).
  - V build (8 flat-shifted row copies of x) split across the sync/scalar
    HWDGE rings and the gpsimd SWDGE ring, sample-outer so sample 0's
    chunks start after ~1/4 of the DMA.
  - ConvT decomposes into 8 phases packed as 128 PSUM rows; chunks are
    region-pure boxes of N=465 (interior) grouped 4-to-a-PSUM-tile
    ("quads", 4 banks).  Scan engines split per quad: ScalarE does 2
    activation ops (Square+Identity w/ accumulator) over the whole quad,
    VectorE does per-bank bn_stats.  Greedy cost balance.
  - Sums/sumsq region-masked, phase-summed via ONES matmul, sync-BN
    all-reduce, then the pooled output (a stride-2 conv with the
    pool-collapsed kernel) is normalized by a single fused affine.
  - Pooled conv runs after the stats matmuls to cover the all-reduce;
    each jchunk uses one PSUM bank with 4 concurrent PE-quadrant matmuls.
"""

import numpy as np

S = 32768          # 32*32*32 flat spatial
SPC = 4            # samples per core
NCORES = 8
PAD = 2048
XCAT = SPC * 3 * S + PAD


# ---------------------------------------------------------------------------
# chunk schedule: (even_region, odd_region, d0, nd, h0, nh, w0, nw)
# regions: r = fd*4 + fh*2 + fw  (f=1 means that dim sits at position 31)
# interior chunks are uniform N=465 so a quad (4 banks) scans as one
# strided activation AP on ScalarE.
# ---------------------------------------------------------------------------
def _chunks():
    ch = []
    for md in range(31):                              # interior, region 0
        ch.append((0, 0, md, 1, 0, 31, 0, 15))
        ch.append((0, 0, md, 1, 0, 31, 15, 15))
    for m0, nm in ((0, 8), (8, 8), (16, 8), (24, 7)):  # w-pair {30,31}
        ch.append((0, 1, m0, nm, 0, 31, 30, 2))
    ch.append((2, 2, 0, 16, 31, 1, 0, 30))            # face h
    ch.append((2, 2, 16, 15, 31, 1, 0, 30))
    ch.append((2, 3, 0, 31, 31, 1, 30, 2))            # face h, w-pair
    ch.append((4, 4, 31, 1, 0, 16, 0, 30))            # face d
    ch.append((4, 4, 31, 1, 16, 15, 0, 30))
    ch.append((4, 5, 31, 1, 0, 31, 30, 2))            # face d, w-pair
    ch.append((6, 6, 31, 1, 31, 1, 0, 30))            # edge dh
    ch.append((6, 7, 31, 1, 31, 1, 30, 2))            # edge dh, w-pair
    return ch


_CH = _chunks()
N_INT = 62          # interior chunk count (all N=465, region 0)

# duos: 2 chunks per 2-bank PSUM tile.  Interior duos are ScalarE-eligible.
_QUADS = []
for i in range(31):
    _QUADS.append([2 * i, 2 * i + 1])      # interior duos (region 0, N=465)
_QUADS.append([62, 63])                    # w-pairs        (DVE)
_QUADS.append([64, 65])                    # w-pairs        (DVE)
_QUADS.append([66, 67])                    # face h         (DVE)
_QUADS.append([69, 70])                    # face d         (DVE)
_QUADS.append([68, 71])                    # face w-pairs   (DVE)
_QUADS.append([72, 73])                    # edge + pair    (DVE)
_N_INT_QUADS = 31                          # duos 0..30 are interior
_FORCED_DVE = tuple(range(31, 37))


def _chunk_n(ci):
    _, _, d0, nd, h0, nh, w0, nw = _CH[ci]
    return nd * nh * nw


# measured scan costs (ns) for engine assignment; the 1.27 factor matches
# VectorE's observed busy time (sem overheads beyond the op-cost model)
def _cost_dve(quad):
    return sum((_chunk_n(ci) * 1.042 + 115) * 1.27 for ci in quad)


def _cost_act(quad):
    n = sum(_chunk_n(ci) for ci in quad)
    return 2 * (n / 1.2 + 480)


def _assign_engines():
    """Greedy balance: forced-DVE duos first, then interior duos to the
    engine with the lower accumulated modeled time."""
    assign = {}
    t_dve = sum(_cost_dve(_QUADS[qi]) for qi in _FORCED_DVE)
    t_act = 0.0
    for qi in range(_N_INT_QUADS):
        cd, ca = _cost_dve(_QUADS[qi]), _cost_act(_QUADS[qi])
        if t_dve + cd <= t_act + ca:
            assign[qi] = "dve"
            t_dve += cd
        else:
            assign[qi] = "act"
            t_act += ca
    for qi in _FORCED_DVE:
        assign[qi] = "dve"
    return assign


_ASSIGN = _assign_engines()
# slot maps: DVE bn_stats slots are per (sample, quad, chunk); ACT accum
# slots per (sample, quad)
_DVE_SLOTS = []      # (s, qi, ci)
_ACT_SLOTS = []      # (s, qi)
for _s in range(SPC):
    for _qi, _quad in enumerate(_QUADS):
        if _ASSIGN[_qi] == "dve":
            for _ci in _quad:
                _DVE_SLOTS.append((_s, _qi, _ci))
        else:
            _ACT_SLOTS.append((_s, _qi))
_NDVE = len(_DVE_SLOTS)
_NACT = len(_ACT_SLOTS)


# ---------------------------------------------------------------------------
# host-side constant construction
# ---------------------------------------------------------------------------
def _f32_to_bf16(a):
    import ml_dtypes
    return np.ascontiguousarray(np.asarray(a, np.float32)).astype(ml_dtypes.bfloat16)


def _host_consts(weight, gamma, beta):
    w = np.asarray(weight, np.float32)            # (3,16,3,3,3)

    # W128[(cin,dd,dh,dw), 16*P + c], P = 4*ed+2*eh+ew
    W128 = np.zeros((24, 128), np.float32)
    for cin in range(3):
        for dd in range(2):
            for dh in range(2):
                for dw in range(2):
                    k = cin * 8 + dd * 4 + dh * 2 + dw
                    for P in range(8):
                        ed, eh, ew = P >> 2 & 1, P >> 1 & 1, P & 1
                        ok, ts = True, []
                        for e, d in ((ed, dd), (eh, dh), (ew, dw)):
                            if e == 0:
                                if d != 0:
                                    ok = False
                                    break
                                ts.append(1)
                            else:
                                ts.append(2 - 2 * d)
                        if ok:
                            W128[k, P * 16:P * 16 + 16] = w[cin, :, ts[0], ts[1], ts[2]]

    # pooled effective kernel
    Phi = np.zeros((3, 3), np.float32)
    Phi[0, 1] = Phi[0, 2] = 1
    Phi[1, :] = 1
    Phi[2, 0] = 1
    Weff = np.einsum("at,bu,gv,nctuv->ncabg", Phi, Phi, Phi, w).astype(np.float32)

    # WPT[(cin,bd,bh,bw), 16*p + c] : pass p=(od,oh,ow), tap s=b+2o
    WPT = np.zeros((24, 128), np.float32)
    for p in range(8):
        od, oh, ow = p >> 2 & 1, p >> 1 & 1, p & 1
        for cin in range(3):
            for bd in range(2):
                for bh in range(2):
                    for bw in range(2):
                        sd, sh, sw = bd + 2 * od, bh + 2 * oh, bw + 2 * ow
                        if 3 in (sd, sh, sw):
                            continue
                        k = cin * 8 + bd * 4 + bh * 2 + bw
                        WPT[k, p * 16:p * 16 + 16] = Weff[cin, :, sd, sh, sw]

    # ones128[16*P + c, 32*s + c] = 1  (phase-sum + broadcast per sample band)
    ONES = np.zeros((128, 128), np.float32)
    for P in range(8):
        for c in range(16):
            for s in range(4):
                ONES[P * 16 + c, 32 * s + c] = 1.0

    # region validity per phase row: MASK[row, r]
    MASK = np.zeros((128, 8), np.float32)
    for P in range(8):
        ed, eh, ew = P >> 2 & 1, P >> 1 & 1, P & 1
        for r in range(8):
            fd, fh, fw = r >> 2 & 1, r >> 1 & 1, r & 1
            if (not fd or ed == 0) and (not fh or eh == 0) and (not fw or ew == 0):
                MASK[P * 16:P * 16 + 16, r] = 1.0

    # DVE bn_stats region weights: 2 cols (even, odd) per DVE slot
    REGW = np.zeros((128, 2 * _NDVE), np.float32)
    for t, (_s, _qi, ci) in enumerate(_DVE_SLOTS):
        er, orr = _CH[ci][0], _CH[ci][1]
        REGW[:, 2 * t] = MASK[:, er]
        REGW[:, 2 * t + 1] = MASK[:, orr]

    # ACT accum region weights (interior quads are region 0)
    AREGW = np.zeros((128, _NACT), np.float32)
    for a, (_s, qi) in enumerate(_ACT_SLOTS):
        er = _CH[_QUADS[qi][0]][0]
        AREGW[:, a] = MASK[:, er]

    GB = np.zeros((128, 3), np.float32)
    for s in range(4):
        GB[32 * s:32 * s + 16, 0] = gamma
        GB[32 * s:32 * s + 16, 1] = beta
        GB[32 * s:32 * s + 16, 2] = gamma / 64.0

    # zero-pad weights to full 32-row PE row-groups: rows 24-31 contribute
    # exactly zero even if the array contracts the whole padded group
    W128p = np.zeros((32, 128), np.float32)
    W128p[:24] = W128
    WPTp = np.zeros((32, 128), np.float32)
    WPTp[:24] = WPT
    return dict(
        w128=_f32_to_bf16(W128p), wpt=_f32_to_bf16(WPTp),
        ones=ONES, regw=REGW, aregw=AREGW, gb=GB,
    )


# ---------------------------------------------------------------------------
# bass kernel builder
# ---------------------------------------------------------------------------
_BUILD_CACHE = {}


def build_nc(n_cores=NCORES):
    if n_cores in _BUILD_CACHE:
        return _BUILD_CACHE[n_cores]
    import concourse.bacc as bacc
    import concourse.tile as tile
    import concourse.mybir as mybir

    f32 = mybir.dt.float32
    bf16 = mybir.dt.bfloat16
    ALU = mybir.AluOpType
    AFT = mybir.ActivationFunctionType
    CNT = float(n_cores * SPC * 63 ** 3)

    nc = bacc.Bacc(
        "TRN2",
        target_bir_lowering=False,
        debug=False,
        num_devices=n_cores,
    )
    xcat = nc.dram_tensor("xcat", [XCAT], bf16, kind="ExternalInput")
    w128d = nc.dram_tensor("w128", [32, 128], bf16, kind="ExternalInput")
    wptd = nc.dram_tensor("wpt", [32, 128], bf16, kind="ExternalInput")
    onesd = nc.dram_tensor("ones", [128, 128], f32, kind="ExternalInput")
    maskd = nc.dram_tensor("regw", [128, 2 * _NDVE], f32, kind="ExternalInput")
    aregwd = nc.dram_tensor("aregw", [128, _NACT], f32, kind="ExternalInput")
    gbd = nc.dram_tensor("gb", [128, 3], f32, kind="ExternalInput")
    outd = nc.dram_tensor("out", [SPC, 16, 3375], bf16, kind="ExternalOutput")

    with tile.TileContext(nc) as tc:
        with (
            tc.tile_pool(name="big", bufs=1) as big,
            tc.tile_pool(name="cst", bufs=1) as cst,
            tc.tile_pool(name="sml", bufs=1) as sml,
            tc.tile_pool(name="dram", bufs=1, space="DRAM") as dram,
        ):
            V = big.tile([128, S], bf16)
            STATS = big.tile([128, _NDVE * 6], f32)
            SCR1 = big.tile([128, 2 * _NDVE], f32)
            SCR2 = big.tile([128, 2 * _NDVE], f32)
            praw = big.tile([128, 3375], f32)
            staged = big.tile([128, 3375], bf16)

            W128t = cst.tile([128, 128], bf16)
            WPTt = cst.tile([128, 128], bf16)
            ONESt = cst.tile([128, 128], f32)
            REGWt = cst.tile([128, 2 * _NDVE], f32)
            AREGWt = cst.tile([128, _NACT], f32)
            ASUM = big.tile([128, _NACT], f32)
            ASQ = big.tile([128, _NACT], f32)
            SCRA = big.tile([128, 2048], f32)
            SSA = sml.tile([128, 2], f32)
            GBt = cst.tile([128, 3], f32)

            SS = sml.tile([128, 2], f32)
            ssb = sml.tile([128, 2], f32)
            gss = sml.tile([128, 2], f32)
            meanT = sml.tile([128, 1], f32)
            ex2T = sml.tile([128, 1], f32)
            varT = sml.tile([128, 1], f32)
            sqT = sml.tile([128, 1], f32)
            invT = sml.tile([128, 1], f32)
            sclT = sml.tile([128, 1], f32)
            tmpT = sml.tile([128, 1], f32)
            biaT = sml.tile([128, 1], f32)

            # ---- constants in (scalar ring: ~5us of issue work, done well
            # before the first scan op needs the ACT sequencer) ----
            for s in range(SPC):
                nc.scalar.dma_start(W128t[32 * s:32 * s + 32, :], w128d[:, :])
                nc.scalar.dma_start(WPTt[32 * s:32 * s + 32, :], wptd[:, :])
            nc.scalar.dma_start(ONESt[:, :], onesd[:, :])
            nc.scalar.dma_start(REGWt[:, :], maskd[:, :])
            nc.scalar.dma_start(AREGWt[:, :], aregwd[:, :])
            nc.scalar.dma_start(GBt[:, :], gbd[:, :])

            # ---- V build: rows (32s + cin*8 + delta) = xcat flat-shifted ----
            # split across sync + scalar HWDGE rings and gpsimd SWDGE ring
            Vv = V.rearrange("(s c k) m -> s c k m", s=4, c=4, k=8)
            # round-robin three DMA rings per half-row so sample 0's leading
            # d-range lands as early as possible (compute pipelines behind)
            # V on sync + gpsimd SWDGE: both issuing sequencers are otherwise
            # idle (scalar-ring issue would block the ACT sequencer, a
            # critical scan engine, until the queue drains).
            rings = [nc.sync, nc.gpsimd]
            ring_i = 0
            H = S // 2
            for s in range(SPC):
                for dd in range(2):
                    for dh in range(2):
                        for dw in range(2):
                            d = dd * 4 + dh * 2 + dw
                            off = s * 3 * S + dd * 1024 + dh * 32 + dw
                            src = xcat[off:off + 3 * S].rearrange("(c m) -> c m", m=S)
                            for h in range(2):
                                rings[ring_i % len(rings)].dma_start(
                                    Vv[s, 0:3, d, h * H:(h + 1) * H],
                                    src[0:3, h * H:(h + 1) * H])
                                ring_i += 1

            V4 = V.rearrange("p (d h w) -> p d h w", h=32, w=32)
            V4p = V.rearrange("p (d dp h hp w wp) -> p dp hp wp d h w",
                              d=16, dp=2, h=16, hp=2, w=16, wp=2)

            # ---- main conv + quad scans, sample-outer for DMA overlap ----
            dve_slot = 0
            act_slot = 0
            with tc.tile_pool(name="psA", bufs=4, space="PSUM") as psA:
                for s in range(SPC):
                    for qi, quad in enumerate(_QUADS):
                        qt = psA.tile([128, 1024], f32, tag="duo")
                        offs = []
                        for b, ci in enumerate(quad):
                            er, orr, d0, nd, h0, nh, w0, nw = _CH[ci]
                            N = nd * nh * nw
                            rhs = V4[32 * s:32 * s + 24,
                                     d0:d0 + nd, h0:h0 + nh, w0:w0 + nw]
                            nc.tensor.matmul(
                                qt[:, 512 * b:512 * b + N],
                                W128t[32 * s:32 * s + 24, :],
                                rhs,
                                start=True, stop=True,
                                tile_position=(32 * s, 0),
                            )
                            offs.append(N)
                        if _ASSIGN[qi] == "act":
                            # uniform N=465 interior chunks: one strided AP
                            N = offs[0]
                            ap = qt.rearrange(
                                "p (b c) -> p b c", b=2)[:, :, 0:N]
                            a = act_slot
                            act_slot += 1
                            nc.scalar.activation(SCRA[:, 0:2 * N], ap,
                                                 AFT.Square,
                                                 accum_out=ASQ[:, a:a + 1])
                            nc.scalar.activation(SCRA[:, 0:2 * N], ap,
                                                 AFT.Identity,
                                                 accum_out=ASUM[:, a:a + 1])
                        else:
                            for b, ci in enumerate(quad):
                                N = offs[b]
                                t = dve_slot
                                dve_slot += 1
                                nc.vector.bn_stats(
                                    STATS[:, 6 * t:6 * t + 6],
                                    qt[:, 512 * b:512 * b + N])

            # ---- stats finalize ----
            st3 = STATS.rearrange("p (n t) -> p n t", t=3)
            counts = st3[:, :, 0]
            means = st3[:, :, 1]
            cvs = st3[:, :, 2]
            nc.vector.tensor_tensor(out=SCR1[:, :], in0=counts, in1=means, op=ALU.mult)
            nc.vector.tensor_tensor(out=SCR2[:, :], in0=SCR1[:, :], in1=means, op=ALU.mult)
            nc.vector.tensor_tensor(out=SCR2[:, :], in0=SCR2[:, :], in1=cvs, op=ALU.add)
            nc.vector.tensor_tensor(out=SCR2[:, :], in0=SCR2[:, :], in1=REGWt[:, :], op=ALU.mult)
            nc.vector.reduce_sum(SS[:, 1:2], SCR2[:, :], axis=mybir.AxisListType.X)
            nc.vector.tensor_tensor(out=SCR1[:, :], in0=SCR1[:, :], in1=REGWt[:, :], op=ALU.mult)
            nc.vector.reduce_sum(SS[:, 0:1], SCR1[:, :], axis=mybir.AxisListType.X)
            # merge ScalarE accumulators
            nc.vector.tensor_tensor(out=SCR1[:, 0:_NACT], in0=ASUM[:, :], in1=AREGWt[:, :], op=ALU.mult)
            nc.vector.reduce_sum(SSA[:, 0:1], SCR1[:, 0:_NACT], axis=mybir.AxisListType.X)
            nc.vector.tensor_tensor(out=SCR1[:, 0:_NACT], in0=ASQ[:, :], in1=AREGWt[:, :], op=ALU.mult)
            nc.vector.reduce_sum(SSA[:, 1:2], SCR1[:, 0:_NACT], axis=mybir.AxisListType.X)
            nc.vector.tensor_tensor(out=SS[:, :], in0=SS[:, :], in1=SSA[:, :], op=ALU.add)

            # phase-sum + broadcast to per-sample channel rows
            psB_cm = tc.tile_pool(name="psB", bufs=2, space="PSUM")
            psB = psB_cm.__enter__()
            pss = psB.tile([128, 2], f32, tag="pss")
            nc.tensor.matmul(pss[:, :], ONESt[:, :], SS[:, :], start=True, stop=True)
            nc.vector.tensor_copy(ssb[:, :], pss[:, :])

            # ---- sync-BN all-reduce across cores ----
            import os
            if n_cores > 1 and not os.environ.get("KERNEL_NO_CC"):
                cin_b = dram.tile([128, 2], f32)
                cout_b = dram.tile([128, 2], f32)
                nc.gpsimd.dma_start(cin_b[:, :], ssb[:, :])
                nc.gpsimd.collective_compute(
                    "AllReduce",
                    ALU.add,
                    replica_groups=[list(range(n_cores))],
                    ins=[cin_b.opt()],
                    outs=[cout_b.opt()],
                )
                nc.gpsimd.dma_start(gss[:, :], cout_b[:, :])
            else:
                nc.vector.tensor_copy(gss[:, :], ssb[:, :])

            # ---- pooled conv after the stats matmuls (covers all-reduce) ----
            jchunks = [(0, 2), (2, 2), (4, 2), (6, 2), (8, 2), (10, 2), (12, 2), (13, 2)]
            for jce, (jd0, njd) in enumerate(jchunks):
                NP = njd * 225
                pchunk = psB.tile([128, 512], f32, tag="pchunk")
                for p in range(8):
                    od, oh, ow = p >> 2 & 1, p >> 1 & 1, p & 1
                    for s in range(SPC):
                        rhs = V4p[32 * s:32 * s + 24, 0, 0, 0,
                                  od + jd0:od + jd0 + njd,
                                  oh:oh + 15, ow:ow + 15]
                        nc.tensor.matmul(
                            pchunk[32 * s:32 * s + 16, 0:NP],
                            WPTt[32 * s:32 * s + 24, 16 * p:16 * p + 16],
                            rhs,
                            start=(p == 0), stop=(p == 7),
                            tile_position=(32 * s, 32 * s),
                        )
                # copies on DVE only: it idles during the all-reduce window
                dst = praw[:, 225 * jd0:225 * jd0 + NP]
                nc.vector.tensor_copy(dst, pchunk[:, 0:NP])

            # ---- finalize scalars (fused) ----
            nc.vector.tensor_scalar_mul(meanT[:, :], gss[:, 0:1], 1.0 / CNT)
            nc.vector.tensor_scalar_mul(ex2T[:, :], gss[:, 1:2], 1.0 / CNT)
            nc.vector.tensor_tensor(out=varT[:, :], in0=meanT[:, :], in1=meanT[:, :], op=ALU.mult)
            nc.vector.tensor_tensor(out=varT[:, :], in0=ex2T[:, :], in1=varT[:, :], op=ALU.subtract)
            nc.vector.tensor_scalar_add(varT[:, :], varT[:, :], 1e-5)
            nc.scalar.activation(sqT[:, :], varT[:, :], AFT.Sqrt)
            nc.vector.reciprocal(invT[:, :], sqT[:, :])
            # scale = inv*gamma/64 ; bias = beta - mean*inv*gamma
            nc.vector.tensor_tensor(out=sclT[:, :], in0=invT[:, :], in1=GBt[:, 2:3], op=ALU.mult)
            nc.vector.tensor_tensor(out=tmpT[:, :], in0=meanT[:, :], in1=invT[:, :], op=ALU.mult)
            nc.vector.tensor_tensor(out=tmpT[:, :], in0=tmpT[:, :], in1=GBt[:, 0:1], op=ALU.mult)
            nc.vector.tensor_tensor(out=biaT[:, :], in0=GBt[:, 1:2], in1=tmpT[:, :], op=ALU.subtract)

            # ---- normalize + out (bf16, host upcasts); split halves so the
            # first out DMAs overlap the second affine half; DMAs on the
            # otherwise-idle sync ring ----
            HJ = 1700
            for h, (c0, c1) in enumerate(((0, HJ), (HJ, 3375))):
                nc.scalar.activation(staged[:, c0:c1], praw[:, c0:c1], AFT.Identity,
                                     bias=biaT[:, 0:1], scale=sclT[:, 0:1])
                for s in range(SPC):
                    nc.sync.dma_start(outd[s][:, c0:c1],
                                      staged[32 * s:32 * s + 16, c0:c1])
            psB_cm.__exit__(None, None, None)

    nc.compile()
    _BUILD_CACHE[n_cores] = nc
    return nc


# ---------------------------------------------------------------------------
# host entry point
# ---------------------------------------------------------------------------
def make_in_maps(x, weight, gamma, beta, n_cores=NCORES):
    x = np.ascontiguousarray(np.asarray(x, np.float32))
    consts = _host_consts(weight, np.asarray(gamma, np.float32), np.asarray(beta, np.float32))
    in_maps = []
    for core in range(n_cores):
        xs = x[core * SPC:(core + 1) * SPC]
        import ml_dtypes
        xc = np.zeros(XCAT, ml_dtypes.bfloat16)
        xc[:SPC * 3 * S] = _f32_to_bf16(xs.reshape(-1))
        in_maps.append({
            "xcat": xc,
            "w128": consts["w128"],
            "wpt": consts["wpt"],
            "ones": consts["ones"],
            "regw": consts["regw"],
            "aregw": consts["aregw"],
            "gb": consts["gb"],
        })
    return in_maps


def _bf16_bits_to_f32(a):
    a = np.asarray(a)
    if a.dtype == np.uint16:
        return (a.astype(np.uint32) << 16).view(np.float32)
    return a.astype(np.float32)


def kernel(x, weight, gamma, beta):
    import sys
    if "/opt/trn_rl_repo" not in sys.path:
        sys.path.insert(0, "/opt/trn_rl_repo")
    from concourse.bass_utils import run_bass_kernel_spmd

    nc = build_nc(NCORES)
    in_maps = make_in_maps(x, weight, gamma, beta, NCORES)
    res = run_bass_kernel_spmd(nc, in_maps, core_ids=list(range(NCORES)))
    outs = [_bf16_bits_to_f32(r["out"]).reshape(SPC, 16, 15, 15, 15)
            for r in res.results]
    return np.concatenate(outs, axis=0)


if __name__ == "__main__":
    import sys
    sys.path.insert(0, "/opt/trn_rl_repo")
    sys.path.insert(0, "/root/problem")
    import reference as ref
    inputs = {k: np.asarray(v) for k, v in ref.setup_inputs().items()}
    out = kernel(**inputs)
    print("out shape", out.shape)


# revision 32
# speedup vs baseline: 1.3489x; 1.3489x over previous
"""Trainium2 Bass kernel for ConvTranspose3d(3->16,k3,s2,p1) + BatchNorm3d(train) + 2x AvgPool3d(2).

Final design (bf16, dual-ring DMA, duo-batched scans; 256us vs 503us baseline):
  - All conv inputs/weights/outputs in bf16 (tolerance 2e-2 >> bf16 ~3e-3).
  - V build (8 flat-shifted row copies of x, half-rows) round-robins the
    sync HWDGE and gpsimd SWDGE rings ONLY: issuing bulk DMAs from the
    scalar ring stalls the ACT *sequencer* (a critical scan engine) until
    the queue drains.  Sample-outer order so compute pipelines behind.
  - ConvT decomposes into 8 phases packed as 128 PSUM rows; chunks are
    region-pure boxes of N=465 (interior) paired 2-to-a-PSUM-tile
    ("duos", 2 banks, bufs=4).  Scan engines split per duo: ScalarE does
    2 activation ops (Square+Identity w/ accumulator) over a strided
    2-bank AP, VectorE does per-bank bn_stats.  Greedy cost balance.
  - Sums/sumsq region-masked, phase-summed via ONES matmul, sync-BN
    all-reduce, then the pooled output (a stride-2 conv with the
    pool-collapsed kernel) is normalized by a fused affine (split halves
    so out DMAs overlap).  Weights zero-padded to 32-row PE groups.
  - Pooled conv runs after the stats matmuls: the ~13us/core SPMD launch
    stagger makes work between collective trigger and result free.
"""

import numpy as np

S = 32768          # 32*32*32 flat spatial
SPC = 4            # samples per core
NCORES = 8
PAD = 2048
XCAT = SPC * 3 * S + PAD


# ---------------------------------------------------------------------------
# chunk schedule: (even_region, odd_region, d0, nd, h0, nh, w0, nw)
# regions: r = fd*4 + fh*2 + fw  (f=1 means that dim sits at position 31)
# interior chunks are uniform N=465 so a quad (4 banks) scans as one
# strided activation AP on ScalarE.
# ---------------------------------------------------------------------------
def _chunks():
    ch = []
    for md in range(31):                              # interior, region 0
        ch.append((0, 0, md, 1, 0, 31, 0, 15))
        ch.append((0, 0, md, 1, 0, 31, 15, 15))
    for m0, nm in ((0, 8), (8, 8), (16, 8), (24, 7)):  # w-pair {30,31}
        ch.append((0, 1, m0, nm, 0, 31, 30, 2))
    ch.append((2, 2, 0, 16, 31, 1, 0, 30))            # face h
    ch.append((2, 2, 16, 15, 31, 1, 0, 30))
    ch.append((2, 3, 0, 31, 31, 1, 30, 2))            # face h, w-pair
    ch.append((4, 4, 31, 1, 0, 16, 0, 30))            # face d
    ch.append((4, 4, 31, 1, 16, 15, 0, 30))
    ch.append((4, 5, 31, 1, 0, 31, 30, 2))            # face d, w-pair
    ch.append((6, 6, 31, 1, 31, 1, 0, 30))            # edge dh
    ch.append((6, 7, 31, 1, 31, 1, 30, 2))            # edge dh, w-pair
    return ch


_CH = _chunks()
N_INT = 62          # interior chunk count (all N=465, region 0)

# duos: 2 chunks per 2-bank PSUM tile.  Interior duos are ScalarE-eligible.
_QUADS = []
for i in range(31):
    _QUADS.append([2 * i, 2 * i + 1])      # interior duos (region 0, N=465)
_QUADS.append([62, 63])                    # w-pairs        (DVE)
_QUADS.append([64, 65])                    # w-pairs        (DVE)
_QUADS.append([66, 67])                    # face h         (DVE)
_QUADS.append([69, 70])                    # face d         (DVE)
_QUADS.append([68, 71])                    # face w-pairs   (DVE)
_QUADS.append([72, 73])                    # edge + pair    (DVE)
_N_INT_QUADS = 31                          # duos 0..30 are interior
_FORCED_DVE = tuple(range(31, 37))


def _chunk_n(ci):
    _, _, d0, nd, h0, nh, w0, nw = _CH[ci]
    return nd * nh * nw


# measured scan costs (ns) for engine assignment
def _cost_dve(quad):
    return sum(_chunk_n(ci) * 1.042 + 115 for ci in quad)


def _cost_act(quad):
    n = sum(_chunk_n(ci) for ci in quad)
    return 2 * (n / 1.2 + 480)


def _assign_engines():
    """Greedy balance: forced-DVE duos first, then interior duos to the
    engine with the lower accumulated modeled time."""
    assign = {}
    t_dve = sum(_cost_dve(_QUADS[qi]) for qi in _FORCED_DVE)
    t_act = 0.0
    for qi in range(_N_INT_QUADS):
        cd, ca = _cost_dve(_QUADS[qi]), _cost_act(_QUADS[qi])
        if t_dve + cd <= t_act + ca:
            assign[qi] = "dve"
            t_dve += cd
        else:
            assign[qi] = "act"
            t_act += ca
    for qi in _FORCED_DVE:
        assign[qi] = "dve"
    return assign


_ASSIGN = _assign_engines()
# slot maps: DVE bn_stats slots are per (sample, quad, chunk); ACT accum
# slots per (sample, quad)
_DVE_SLOTS = []      # (s, qi, ci)
_ACT_SLOTS = []      # (s, qi)
for _s in range(SPC):
    for _qi, _quad in enumerate(_QUADS):
        if _ASSIGN[_qi] == "dve":
            for _ci in _quad:
                _DVE_SLOTS.append((_s, _qi, _ci))
        else:
            _ACT_SLOTS.append((_s, _qi))
_NDVE = len(_DVE_SLOTS)
_NACT = len(_ACT_SLOTS)


# ---------------------------------------------------------------------------
# host-side constant construction
# ---------------------------------------------------------------------------
def _f32_to_bf16(a):
    import ml_dtypes
    return np.ascontiguousarray(np.asarray(a, np.float32)).astype(ml_dtypes.bfloat16)


def _host_consts(weight, gamma, beta):
    w = np.asarray(weight, np.float32)            # (3,16,3,3,3)

    # W128[(cin,dd,dh,dw), 16*P + c], P = 4*ed+2*eh+ew
    W128 = np.zeros((24, 128), np.float32)
    for cin in range(3):
        for dd in range(2):
            for dh in range(2):
                for dw in range(2):
                    k = cin * 8 + dd * 4 + dh * 2 + dw
                    for P in range(8):
                        ed, eh, ew = P >> 2 & 1, P >> 1 & 1, P & 1
                        ok, ts = True, []
                        for e, d in ((ed, dd), (eh, dh), (ew, dw)):
                            if e == 0:
                                if d != 0:
                                    ok = False
                                    break
                                ts.append(1)
                            else:
                                ts.append(2 - 2 * d)
                        if ok:
                            W128[k, P * 16:P * 16 + 16] = w[cin, :, ts[0], ts[1], ts[2]]

    # pooled effective kernel
    Phi = np.zeros((3, 3), np.float32)
    Phi[0, 1] = Phi[0, 2] = 1
    Phi[1, :] = 1
    Phi[2, 0] = 1
    Weff = np.einsum("at,bu,gv,nctuv->ncabg", Phi, Phi, Phi, w).astype(np.float32)

    # WPT[(cin,bd,bh,bw), 16*p + c] : pass p=(od,oh,ow), tap s=b+2o
    WPT = np.zeros((24, 128), np.float32)
    for p in range(8):
        od, oh, ow = p >> 2 & 1, p >> 1 & 1, p & 1
        for cin in range(3):
            for bd in range(2):
                for bh in range(2):
                    for bw in range(2):
                        sd, sh, sw = bd + 2 * od, bh + 2 * oh, bw + 2 * ow
                        if 3 in (sd, sh, sw):
                            continue
                        k = cin * 8 + bd * 4 + bh * 2 + bw
                        WPT[k, p * 16:p * 16 + 16] = Weff[cin, :, sd, sh, sw]

    # ones128[16*P + c, 32*s + c] = 1  (phase-sum + broadcast per sample band)
    ONES = np.zeros((128, 128), np.float32)
    for P in range(8):
        for c in range(16):
            for s in range(4):
                ONES[P * 16 + c, 32 * s + c] = 1.0

    # region validity per phase row: MASK[row, r]
    MASK = np.zeros((128, 8), np.float32)
    for P in range(8):
        ed, eh, ew = P >> 2 & 1, P >> 1 & 1, P & 1
        for r in range(8):
            fd, fh, fw = r >> 2 & 1, r >> 1 & 1, r & 1
            if (not fd or ed == 0) and (not fh or eh == 0) and (not fw or ew == 0):
                MASK[P * 16:P * 16 + 16, r] = 1.0

    # DVE bn_stats region weights: 2 cols (even, odd) per DVE slot
    REGW = np.zeros((128, 2 * _NDVE), np.float32)
    for t, (_s, _qi, ci) in enumerate(_DVE_SLOTS):
        er, orr = _CH[ci][0], _CH[ci][1]
        REGW[:, 2 * t] = MASK[:, er]
        REGW[:, 2 * t + 1] = MASK[:, orr]

    # ACT accum region weights (interior quads are region 0)
    AREGW = np.zeros((128, _NACT), np.float32)
    for a, (_s, qi) in enumerate(_ACT_SLOTS):
        er = _CH[_QUADS[qi][0]][0]
        AREGW[:, a] = MASK[:, er]

    GB = np.zeros((128, 3), np.float32)
    for s in range(4):
        GB[32 * s:32 * s + 16, 0] = gamma
        GB[32 * s:32 * s + 16, 1] = beta
        GB[32 * s:32 * s + 16, 2] = gamma / 64.0

    # zero-pad weights to full 32-row PE row-groups: rows 24-31 contribute
    # exactly zero even if the array contracts the whole padded group
    W128p = np.zeros((32, 128), np.float32)
    W128p[:24] = W128
    WPTp = np.zeros((32, 128), np.float32)
    WPTp[:24] = WPT
    return dict(
        w128=_f32_to_bf16(W128p), wpt=_f32_to_bf16(WPTp),
        ones=ONES, regw=REGW, aregw=AREGW, gb=GB,
    )


# ---------------------------------------------------------------------------
# bass kernel builder
# ---------------------------------------------------------------------------
_BUILD_CACHE = {}


def build_nc(n_cores=NCORES):
    if n_cores in _BUILD_CACHE:
        return _BUILD_CACHE[n_cores]
    import concourse.bacc as bacc
    import concourse.tile as tile
    import concourse.mybir as mybir

    f32 = mybir.dt.float32
    bf16 = mybir.dt.bfloat16
    ALU = mybir.AluOpType
    AFT = mybir.ActivationFunctionType
    CNT = float(n_cores * SPC * 63 ** 3)

    nc = bacc.Bacc(
        "TRN2",
        target_bir_lowering=False,
        debug=False,
        num_devices=n_cores,
    )
    xcat = nc.dram_tensor("xcat", [XCAT], bf16, kind="ExternalInput")
    w128d = nc.dram_tensor("w128", [32, 128], bf16, kind="ExternalInput")
    wptd = nc.dram_tensor("wpt", [32, 128], bf16, kind="ExternalInput")
    onesd = nc.dram_tensor("ones", [128, 128], f32, kind="ExternalInput")
    maskd = nc.dram_tensor("regw", [128, 2 * _NDVE], f32, kind="ExternalInput")
    aregwd = nc.dram_tensor("aregw", [128, _NACT], f32, kind="ExternalInput")
    gbd = nc.dram_tensor("gb", [128, 3], f32, kind="ExternalInput")
    outd = nc.dram_tensor("out", [SPC, 16, 3375], bf16, kind="ExternalOutput")

    with tile.TileContext(nc) as tc:
        with (
            tc.tile_pool(name="big", bufs=1) as big,
            tc.tile_pool(name="cst", bufs=1) as cst,
            tc.tile_pool(name="sml", bufs=1) as sml,
            tc.tile_pool(name="dram", bufs=1, space="DRAM") as dram,
        ):
            V = big.tile([128, S], bf16)
            STATS = big.tile([128, _NDVE * 6], f32)
            SCR1 = big.tile([128, 2 * _NDVE], f32)
            SCR2 = big.tile([128, 2 * _NDVE], f32)
            praw = big.tile([128, 3375], f32)
            staged = big.tile([128, 3375], bf16)

            W128t = cst.tile([128, 128], bf16)
            WPTt = cst.tile([128, 128], bf16)
            ONESt = cst.tile([128, 128], f32)
            REGWt = cst.tile([128, 2 * _NDVE], f32)
            AREGWt = cst.tile([128, _NACT], f32)
            ASUM = big.tile([128, _NACT], f32)
            ASQ = big.tile([128, _NACT], f32)
            SCRA = big.tile([128, 2048], f32)
            SSA = sml.tile([128, 2], f32)
            GBt = cst.tile([128, 3], f32)

            SS = sml.tile([128, 2], f32)
            ssb = sml.tile([128, 2], f32)
            gss = sml.tile([128, 2], f32)
            meanT = sml.tile([128, 1], f32)
            ex2T = sml.tile([128, 1], f32)
            varT = sml.tile([128, 1], f32)
            sqT = sml.tile([128, 1], f32)
            invT = sml.tile([128, 1], f32)
            sclT = sml.tile([128, 1], f32)
            tmpT = sml.tile([128, 1], f32)
            biaT = sml.tile([128, 1], f32)

            # ---- constants in (scalar ring: ~5us of issue work, done well
            # before the first scan op needs the ACT sequencer) ----
            for s in range(SPC):
                nc.scalar.dma_start(W128t[32 * s:32 * s + 32, :], w128d[:, :])
                nc.scalar.dma_start(WPTt[32 * s:32 * s + 32, :], wptd[:, :])
            nc.scalar.dma_start(ONESt[:, :], onesd[:, :])
            nc.scalar.dma_start(REGWt[:, :], maskd[:, :])
            nc.scalar.dma_start(AREGWt[:, :], aregwd[:, :])
            nc.scalar.dma_start(GBt[:, :], gbd[:, :])

            # ---- V build: rows (32s + cin*8 + delta) = xcat flat-shifted ----
            # split across sync + scalar HWDGE rings and gpsimd SWDGE ring
            Vv = V.rearrange("(s c k) m -> s c k m", s=4, c=4, k=8)
            # round-robin three DMA rings per half-row so sample 0's leading
            # d-range lands as early as possible (compute pipelines behind)
            # V on sync + gpsimd SWDGE: both issuing sequencers are otherwise
            # idle (scalar-ring issue would block the ACT sequencer, a
            # critical scan engine, until the queue drains).
            rings = [nc.sync, nc.gpsimd]
            ring_i = 0
            H = S // 2
            for s in range(SPC):
                for dd in range(2):
                    for dh in range(2):
                        for dw in range(2):
                            d = dd * 4 + dh * 2 + dw
                            off = s * 3 * S + dd * 1024 + dh * 32 + dw
                            src = xcat[off:off + 3 * S].rearrange("(c m) -> c m", m=S)
                            for h in range(2):
                                rings[ring_i % len(rings)].dma_start(
                                    Vv[s, 0:3, d, h * H:(h + 1) * H],
                                    src[0:3, h * H:(h + 1) * H])
                                ring_i += 1

            V4 = V.rearrange("p (d h w) -> p d h w", h=32, w=32)
            V4p = V.rearrange("p (d dp h hp w wp) -> p dp hp wp d h w",
                              d=16, dp=2, h=16, hp=2, w=16, wp=2)

            # ---- main conv + quad scans, sample-outer for DMA overlap ----
            dve_slot = 0
            act_slot = 0
            with tc.tile_pool(name="psA", bufs=4, space="PSUM") as psA:
                for s in range(SPC):
                    for qi, quad in enumerate(_QUADS):
                        qt = psA.tile([128, 1024], f32, tag="duo")
                        offs = []
                        for b, ci in enumerate(quad):
                            er, orr, d0, nd, h0, nh, w0, nw = _CH[ci]
                            N = nd * nh * nw
                            rhs = V4[32 * s:32 * s + 24,
                                     d0:d0 + nd, h0:h0 + nh, w0:w0 + nw]
                            nc.tensor.matmul(
                                qt[:, 512 * b:512 * b + N],
                                W128t[32 * s:32 * s + 24, :],
                                rhs,
                                start=True, stop=True,
                                tile_position=(32 * s, 0),
                            )
                            offs.append(N)
                        if _ASSIGN[qi] == "act":
                            # uniform N=465 interior chunks: one strided AP
                            N = offs[0]
                            ap = qt.rearrange(
                                "p (b c) -> p b c", b=2)[:, :, 0:N]
                            a = act_slot
                            act_slot += 1
                            nc.scalar.activation(SCRA[:, 0:2 * N], ap,
                                                 AFT.Square,
                                                 accum_out=ASQ[:, a:a + 1])
                            nc.scalar.activation(SCRA[:, 0:2 * N], ap,
                                                 AFT.Identity,
                                                 accum_out=ASUM[:, a:a + 1])
                        else:
                            for b, ci in enumerate(quad):
                                N = offs[b]
                                t = dve_slot
                                dve_slot += 1
                                nc.vector.bn_stats(
                                    STATS[:, 6 * t:6 * t + 6],
                                    qt[:, 512 * b:512 * b + N])

            # ---- stats finalize ----
            st3 = STATS.rearrange("p (n t) -> p n t", t=3)
            counts = st3[:, :, 0]
            means = st3[:, :, 1]
            cvs = st3[:, :, 2]
            nc.vector.tensor_tensor(out=SCR1[:, :], in0=counts, in1=means, op=ALU.mult)
            nc.vector.tensor_tensor(out=SCR2[:, :], in0=SCR1[:, :], in1=means, op=ALU.mult)
            nc.vector.tensor_tensor(out=SCR2[:, :], in0=SCR2[:, :], in1=cvs, op=ALU.add)
            nc.vector.tensor_tensor(out=SCR2[:, :], in0=SCR2[:, :], in1=REGWt[:, :], op=ALU.mult)
            nc.vector.reduce_sum(SS[:, 1:2], SCR2[:, :], axis=mybir.AxisListType.X)
            nc.vector.tensor_tensor(out=SCR1[:, :], in0=SCR1[:, :], in1=REGWt[:, :], op=ALU.mult)
            nc.vector.reduce_sum(SS[:, 0:1], SCR1[:, :], axis=mybir.AxisListType.X)
            # merge ScalarE accumulators
            nc.vector.tensor_tensor(out=SCR1[:, 0:_NACT], in0=ASUM[:, :], in1=AREGWt[:, :], op=ALU.mult)
            nc.vector.reduce_sum(SSA[:, 0:1], SCR1[:, 0:_NACT], axis=mybir.AxisListType.X)
            nc.vector.tensor_tensor(out=SCR1[:, 0:_NACT], in0=ASQ[:, :], in1=AREGWt[:, :], op=ALU.mult)
            nc.vector.reduce_sum(SSA[:, 1:2], SCR1[:, 0:_NACT], axis=mybir.AxisListType.X)
            nc.vector.tensor_tensor(out=SS[:, :], in0=SS[:, :], in1=SSA[:, :], op=ALU.add)

            # phase-sum + broadcast to per-sample channel rows
            psB_cm = tc.tile_pool(name="psB", bufs=2, space="PSUM")
            psB = psB_cm.__enter__()
            pss = psB.tile([128, 2], f32, tag="pss")
            nc.tensor.matmul(pss[:, :], ONESt[:, :], SS[:, :], start=True, stop=True)
            nc.vector.tensor_copy(ssb[:, :], pss[:, :])

            # ---- sync-BN all-reduce across cores ----
            import os
            if n_cores > 1 and not os.environ.get("KERNEL_NO_CC"):
                cin_b = dram.tile([128, 2], f32)
                cout_b = dram.tile([128, 2], f32)
                nc.gpsimd.dma_start(cin_b[:, :], ssb[:, :])
                nc.gpsimd.collective_compute(
                    "AllReduce",
                    ALU.add,
                    replica_groups=[list(range(n_cores))],
                    ins=[cin_b.opt()],
                    outs=[cout_b.opt()],
                )
                nc.gpsimd.dma_start(gss[:, :], cout_b[:, :])
            else:
                nc.vector.tensor_copy(gss[:, :], ssb[:, :])

            # ---- pooled conv after the stats matmuls (covers all-reduce) ----
            jchunks = [(0, 2), (2, 2), (4, 2), (6, 2), (8, 2), (10, 2), (12, 2), (13, 2)]
            for jce, (jd0, njd) in enumerate(jchunks):
                NP = njd * 225
                pchunk = psB.tile([128, 512], f32, tag="pchunk")
                for p in range(8):
                    od, oh, ow = p >> 2 & 1, p >> 1 & 1, p & 1
                    for s in range(SPC):
                        rhs = V4p[32 * s:32 * s + 24, 0, 0, 0,
                                  od + jd0:od + jd0 + njd,
                                  oh:oh + 15, ow:ow + 15]
                        nc.tensor.matmul(
                            pchunk[32 * s:32 * s + 16, 0:NP],
                            WPTt[32 * s:32 * s + 24, 16 * p:16 * p + 16],
                            rhs,
                            start=(p == 0), stop=(p == 7),
                            tile_position=(32 * s, 32 * s),
                        )
                # copies on DVE only: it idles during the all-reduce window
                dst = praw[:, 225 * jd0:225 * jd0 + NP]
                nc.vector.tensor_copy(dst, pchunk[:, 0:NP])

            # ---- finalize scalars (fused) ----
            nc.vector.tensor_scalar_mul(meanT[:, :], gss[:, 0:1], 1.0 / CNT)
            nc.vector.tensor_scalar_mul(ex2T[:, :], gss[:, 1:2], 1.0 / CNT)
            nc.vector.tensor_tensor(out=varT[:, :], in0=meanT[:, :], in1=meanT[:, :], op=ALU.mult)
            nc.vector.tensor_tensor(out=varT[:, :], in0=ex2T[:, :], in1=varT[:, :], op=ALU.subtract)
            nc.vector.tensor_scalar_add(varT[:, :], varT[:, :], 1e-5)
            nc.scalar.activation(sqT[:, :], varT[:, :], AFT.Sqrt)
            nc.vector.reciprocal(invT[:, :], sqT[:, :])
            # scale = inv*gamma/64 ; bias = beta - mean*inv*gamma
            nc.vector.tensor_tensor(out=sclT[:, :], in0=invT[:, :], in1=GBt[:, 2:3], op=ALU.mult)
            nc.vector.tensor_tensor(out=tmpT[:, :], in0=meanT[:, :], in1=invT[:, :], op=ALU.mult)
            nc.vector.tensor_tensor(out=tmpT[:, :], in0=tmpT[:, :], in1=GBt[:, 0:1], op=ALU.mult)
            nc.vector.tensor_tensor(out=biaT[:, :], in0=GBt[:, 1:2], in1=tmpT[:, :], op=ALU.subtract)

            # ---- normalize + out (bf16, host upcasts); split halves so the
            # first out DMAs overlap the second affine half; DMAs on the
            # otherwise-idle sync ring ----
            HJ = 1700
            for h, (c0, c1) in enumerate(((0, HJ), (HJ, 3375))):
                nc.scalar.activation(staged[:, c0:c1], praw[:, c0:c1], AFT.Identity,
                                     bias=biaT[:, 0:1], scale=sclT[:, 0:1])
                for s in range(SPC):
                    nc.sync.dma_start(outd[s][:, c0:c1],
                                      staged[32 * s:32 * s + 16, c0:c1])
            psB_cm.__exit__(None, None, None)

    nc.compile()
    _BUILD_CACHE[n_cores] = nc
    return nc


# ---------------------------------------------------------------------------
# host entry point
# ---------------------------------------------------------------------------
def make_in_maps(x, weight, gamma, beta, n_cores=NCORES):
    x = np.ascontiguousarray(np.asarray(x, np.float32))
    consts = _host_consts(weight, np.asarray(gamma, np.float32), np.asarray(beta, np.float32))
    in_maps = []
    for core in range(n_cores):
        xs = x[core * SPC:(core + 1) * SPC]
        import ml_dtypes
        xc = np.zeros(XCAT, ml_dtypes.bfloat16)
        xc[:SPC * 3 * S] = _f32_to_bf16(xs.reshape(-1))
        in_maps.append({
            "xcat": xc,
            "w128": consts["w128"],
            "wpt": consts["wpt"],
            "ones": consts["ones"],
            "regw": consts["regw"],
            "aregw": consts["aregw"],
            "gb": consts["gb"],
        })
    return in_maps


def _bf16_bits_to_f32(a):
    a = np.asarray(a)
    if a.dtype == np.uint16:
        return (a.astype(np.uint32) << 16).view(np.float32)
    return a.astype(np.float32)


def kernel(x, weight, gamma, beta):
    import sys
    if "/opt/trn_rl_repo" not in sys.path:
        sys.path.insert(0, "/opt/trn_rl_repo")
    from concourse.bass_utils import run_bass_kernel_spmd

    nc = build_nc(NCORES)
    in_maps = make_in_maps(x, weight, gamma, beta, NCORES)
    res = run_bass_kernel_spmd(nc, in_maps, core_ids=list(range(NCORES)))
    outs = [_bf16_bits_to_f32(r["out"]).reshape(SPC, 16, 15, 15, 15)
            for r in res.results]
    return np.concatenate(outs, axis=0)


if __name__ == "__main__":
    import sys
    sys.path.insert(0, "/opt/trn_rl_repo")
    sys.path.insert(0, "/root/problem")
    import reference as ref
    inputs = {k: np.asarray(v) for k, v in ref.setup_inputs().items()}
    out = kernel(**inputs)
    print("out shape", out.shape)


# revision 34
# speedup vs baseline: 1.3575x; 1.0064x over previous
"""Trainium2 Bass kernel for ConvTranspose3d(3->16,k3,s2,p1) + BatchNorm3d(train) + 2x AvgPool3d(2).

Final design (bf16, dual-ring DMA, duo-batched scans; 256us vs 503us baseline):
  - All conv inputs/weights/outputs in bf16 (tolerance 2e-2 >> bf16 ~3e-3).
  - V build (8 flat-shifted row copies of x, half-rows) round-robins the
    sync HWDGE and gpsimd SWDGE rings ONLY: issuing bulk DMAs from the
    scalar ring stalls the ACT *sequencer* (a critical scan engine) until
    the queue drains.  Sample-outer order so compute pipelines behind.
  - ConvT decomposes into 8 phases packed as 128 PSUM rows; chunks are
    region-pure boxes of N=465 (interior) paired 2-to-a-PSUM-tile
    ("duos", 2 banks, bufs=4).  Scan engines split per duo: ScalarE does
    2 activation ops (Square+Identity w/ accumulator) over a strided
    2-bank AP, VectorE does per-bank bn_stats.  Greedy cost balance.
  - Sums/sumsq region-masked, phase-summed via ONES matmul, sync-BN
    all-reduce, then the pooled output (a stride-2 conv with the
    pool-collapsed kernel) is normalized by a fused affine (split halves
    so out DMAs overlap).  Weights zero-padded to 32-row PE groups.
  - Pooled conv runs after the stats matmuls: the ~13us/core SPMD launch
    stagger makes work between collective trigger and result free.
"""

import numpy as np

S = 32768          # 32*32*32 flat spatial
SPC = 4            # samples per core
NCORES = 8
PAD = 2048
XCAT = SPC * 3 * S + PAD


# ---------------------------------------------------------------------------
# chunk schedule: (even_region, odd_region, d0, nd, h0, nh, w0, nw)
# regions: r = fd*4 + fh*2 + fw  (f=1 means that dim sits at position 31)
# interior chunks are uniform N=465 so a quad (4 banks) scans as one
# strided activation AP on ScalarE.
# ---------------------------------------------------------------------------
def _chunks():
    ch = []
    for md in range(31):                              # interior, region 0
        ch.append((0, 0, md, 1, 0, 31, 0, 15))
        ch.append((0, 0, md, 1, 0, 31, 15, 15))
    for m0, nm in ((0, 8), (8, 8), (16, 8), (24, 7)):  # w-pair {30,31}
        ch.append((0, 1, m0, nm, 0, 31, 30, 2))
    ch.append((2, 2, 0, 16, 31, 1, 0, 30))            # face h
    ch.append((2, 2, 16, 15, 31, 1, 0, 30))
    ch.append((2, 3, 0, 31, 31, 1, 30, 2))            # face h, w-pair
    ch.append((4, 4, 31, 1, 0, 16, 0, 30))            # face d
    ch.append((4, 4, 31, 1, 16, 15, 0, 30))
    ch.append((4, 5, 31, 1, 0, 31, 30, 2))            # face d, w-pair
    ch.append((6, 6, 31, 1, 31, 1, 0, 30))            # edge dh
    ch.append((6, 7, 31, 1, 31, 1, 30, 2))            # edge dh, w-pair
    return ch


_CH = _chunks()
N_INT = 62          # interior chunk count (all N=465, region 0)

# duos: 2 chunks per 2-bank PSUM tile.  Interior duos are ScalarE-eligible.
_QUADS = []
for i in range(31):
    _QUADS.append([2 * i, 2 * i + 1])      # interior duos (region 0, N=465)
_QUADS.append([62, 63])                    # w-pairs        (DVE)
_QUADS.append([64, 65])                    # w-pairs        (DVE)
_QUADS.append([66, 67])                    # face h         (DVE)
_QUADS.append([69, 70])                    # face d         (DVE)
_QUADS.append([68, 71])                    # face w-pairs   (DVE)
_QUADS.append([72, 73])                    # edge + pair    (DVE)
_N_INT_QUADS = 31                          # duos 0..30 are interior
_FORCED_DVE = tuple(range(31, 37))


def _chunk_n(ci):
    _, _, d0, nd, h0, nh, w0, nw = _CH[ci]
    return nd * nh * nw


# measured scan costs (ns) for engine assignment
def _cost_dve(quad):
    return sum(_chunk_n(ci) * 1.042 + 115 for ci in quad)


def _cost_act(quad):
    n = sum(_chunk_n(ci) for ci in quad)
    return 2 * (n / 1.2 + 480)


# 11-of-31 interior duos to ScalarE, evenly spaced so neither engine sees
# long same-engine runs (greedy clustering starved ScalarE ~6us at each
# sample boundary); forced-DVE duos spliced mid-sequence for the same reason
_ACT_SET = frozenset((0, 3, 6, 8, 11, 14, 17, 20, 23, 25, 28))
_ASSIGN = {qi: ("act" if qi in _ACT_SET else "dve") for qi in range(len(_QUADS))}
_ORDER = (list(range(0, 11)) + [31, 32] + list(range(11, 21)) + [33, 34]
          + list(range(21, 31)) + [35, 36])

# slot maps: DVE bn_stats slots are per (sample, quad, chunk); ACT accum
# slots per (sample, quad) - both in emission order
_DVE_SLOTS = []      # (s, qi, ci)
_ACT_SLOTS = []      # (s, qi)
for _s in range(SPC):
    for _qi in _ORDER:
        if _ASSIGN[_qi] == "dve":
            for _ci in _QUADS[_qi]:
                _DVE_SLOTS.append((_s, _qi, _ci))
        else:
            _ACT_SLOTS.append((_s, _qi))
_NDVE = len(_DVE_SLOTS)
_NACT = len(_ACT_SLOTS)


# ---------------------------------------------------------------------------
# host-side constant construction
# ---------------------------------------------------------------------------
def _f32_to_bf16(a):
    import ml_dtypes
    return np.ascontiguousarray(np.asarray(a, np.float32)).astype(ml_dtypes.bfloat16)


def _host_consts(weight, gamma, beta):
    w = np.asarray(weight, np.float32)            # (3,16,3,3,3)

    # W128[(cin,dd,dh,dw), 16*P + c], P = 4*ed+2*eh+ew
    W128 = np.zeros((24, 128), np.float32)
    for cin in range(3):
        for dd in range(2):
            for dh in range(2):
                for dw in range(2):
                    k = cin * 8 + dd * 4 + dh * 2 + dw
                    for P in range(8):
                        ed, eh, ew = P >> 2 & 1, P >> 1 & 1, P & 1
                        ok, ts = True, []
                        for e, d in ((ed, dd), (eh, dh), (ew, dw)):
                            if e == 0:
                                if d != 0:
                                    ok = False
                                    break
                                ts.append(1)
                            else:
                                ts.append(2 - 2 * d)
                        if ok:
                            W128[k, P * 16:P * 16 + 16] = w[cin, :, ts[0], ts[1], ts[2]]

    # pooled effective kernel
    Phi = np.zeros((3, 3), np.float32)
    Phi[0, 1] = Phi[0, 2] = 1
    Phi[1, :] = 1
    Phi[2, 0] = 1
    Weff = np.einsum("at,bu,gv,nctuv->ncabg", Phi, Phi, Phi, w).astype(np.float32)

    # WPT[(cin,bd,bh,bw), 16*p + c] : pass p=(od,oh,ow), tap s=b+2o
    WPT = np.zeros((24, 128), np.float32)
    for p in range(8):
        od, oh, ow = p >> 2 & 1, p >> 1 & 1, p & 1
        for cin in range(3):
            for bd in range(2):
                for bh in range(2):
                    for bw in range(2):
                        sd, sh, sw = bd + 2 * od, bh + 2 * oh, bw + 2 * ow
                        if 3 in (sd, sh, sw):
                            continue
                        k = cin * 8 + bd * 4 + bh * 2 + bw
                        WPT[k, p * 16:p * 16 + 16] = Weff[cin, :, sd, sh, sw]

    # ones128[16*P + c, 32*s + c] = 1  (phase-sum + broadcast per sample band)
    ONES = np.zeros((128, 128), np.float32)
    for P in range(8):
        for c in range(16):
            for s in range(4):
                ONES[P * 16 + c, 32 * s + c] = 1.0

    # region validity per phase row: MASK[row, r]
    MASK = np.zeros((128, 8), np.float32)
    for P in range(8):
        ed, eh, ew = P >> 2 & 1, P >> 1 & 1, P & 1
        for r in range(8):
            fd, fh, fw = r >> 2 & 1, r >> 1 & 1, r & 1
            if (not fd or ed == 0) and (not fh or eh == 0) and (not fw or ew == 0):
                MASK[P * 16:P * 16 + 16, r] = 1.0

    # DVE bn_stats region weights: 2 cols (even, odd) per DVE slot
    REGW = np.zeros((128, 2 * _NDVE), np.float32)
    for t, (_s, _qi, ci) in enumerate(_DVE_SLOTS):
        er, orr = _CH[ci][0], _CH[ci][1]
        REGW[:, 2 * t] = MASK[:, er]
        REGW[:, 2 * t + 1] = MASK[:, orr]

    # ACT accum region weights (interior quads are region 0)
    AREGW = np.zeros((128, _NACT), np.float32)
    for a, (_s, qi) in enumerate(_ACT_SLOTS):
        er = _CH[_QUADS[qi][0]][0]
        AREGW[:, a] = MASK[:, er]

    GB = np.zeros((128, 3), np.float32)
    for s in range(4):
        GB[32 * s:32 * s + 16, 0] = gamma
        GB[32 * s:32 * s + 16, 1] = beta
        GB[32 * s:32 * s + 16, 2] = gamma / 64.0

    # zero-pad weights to full 32-row PE row-groups: rows 24-31 contribute
    # exactly zero even if the array contracts the whole padded group
    W128p = np.zeros((32, 128), np.float32)
    W128p[:24] = W128
    WPTp = np.zeros((32, 128), np.float32)
    WPTp[:24] = WPT
    return dict(
        w128=_f32_to_bf16(W128p), wpt=_f32_to_bf16(WPTp),
        ones=ONES, regw=REGW, aregw=AREGW, gb=GB,
    )


# ---------------------------------------------------------------------------
# bass kernel builder
# ---------------------------------------------------------------------------
_BUILD_CACHE = {}


def build_nc(n_cores=NCORES):
    if n_cores in _BUILD_CACHE:
        return _BUILD_CACHE[n_cores]
    import concourse.bacc as bacc
    import concourse.tile as tile
    import concourse.mybir as mybir

    f32 = mybir.dt.float32
    bf16 = mybir.dt.bfloat16
    ALU = mybir.AluOpType
    AFT = mybir.ActivationFunctionType
    CNT = float(n_cores * SPC * 63 ** 3)

    nc = bacc.Bacc(
        "TRN2",
        target_bir_lowering=False,
        debug=False,
        num_devices=n_cores,
    )
    xcat = nc.dram_tensor("xcat", [XCAT], bf16, kind="ExternalInput")
    w128d = nc.dram_tensor("w128", [32, 128], bf16, kind="ExternalInput")
    wptd = nc.dram_tensor("wpt", [32, 128], bf16, kind="ExternalInput")
    onesd = nc.dram_tensor("ones", [128, 128], f32, kind="ExternalInput")
    maskd = nc.dram_tensor("regw", [128, 2 * _NDVE], f32, kind="ExternalInput")
    aregwd = nc.dram_tensor("aregw", [128, _NACT], f32, kind="ExternalInput")
    gbd = nc.dram_tensor("gb", [128, 3], f32, kind="ExternalInput")
    outd = nc.dram_tensor("out", [SPC, 16, 3375], bf16, kind="ExternalOutput")

    with tile.TileContext(nc) as tc:
        with (
            tc.tile_pool(name="big", bufs=1) as big,
            tc.tile_pool(name="cst", bufs=1) as cst,
            tc.tile_pool(name="sml", bufs=1) as sml,
            tc.tile_pool(name="dram", bufs=1, space="DRAM") as dram,
        ):
            V = big.tile([128, S], bf16)
            STATS = big.tile([128, _NDVE * 6], f32)
            SCR1 = big.tile([128, 2 * _NDVE], f32)
            SCR2 = big.tile([128, 2 * _NDVE], f32)
            praw = big.tile([128, 3375], f32)
            staged = big.tile([128, 3375], bf16)

            W128t = cst.tile([128, 128], bf16)
            WPTt = cst.tile([128, 128], bf16)
            ONESt = cst.tile([128, 128], f32)
            REGWt = cst.tile([128, 2 * _NDVE], f32)
            AREGWt = cst.tile([128, _NACT], f32)
            ASUM = big.tile([128, _NACT], f32)
            ASQ = big.tile([128, _NACT], f32)
            SCRA = big.tile([128, 2048], f32)
            SSA = sml.tile([128, 2], f32)
            GBt = cst.tile([128, 3], f32)

            SS = sml.tile([128, 2], f32)
            ssb = sml.tile([128, 2], f32)
            gss = sml.tile([128, 2], f32)
            meanT = sml.tile([128, 1], f32)
            ex2T = sml.tile([128, 1], f32)
            varT = sml.tile([128, 1], f32)
            sqT = sml.tile([128, 1], f32)
            invT = sml.tile([128, 1], f32)
            sclT = sml.tile([128, 1], f32)
            tmpT = sml.tile([128, 1], f32)
            biaT = sml.tile([128, 1], f32)

            # ---- constants in (scalar ring: ~5us of issue work, done well
            # before the first scan op needs the ACT sequencer) ----
            for s in range(SPC):
                nc.scalar.dma_start(W128t[32 * s:32 * s + 32, :], w128d[:, :])
                nc.scalar.dma_start(WPTt[32 * s:32 * s + 32, :], wptd[:, :])
            nc.scalar.dma_start(ONESt[:, :], onesd[:, :])
            nc.scalar.dma_start(REGWt[:, :], maskd[:, :])
            nc.scalar.dma_start(AREGWt[:, :], aregwd[:, :])
            nc.scalar.dma_start(GBt[:, :], gbd[:, :])

            # ---- V build: rows (32s + cin*8 + delta) = xcat flat-shifted ----
            # split across sync + scalar HWDGE rings and gpsimd SWDGE ring
            Vv = V.rearrange("(s c k) m -> s c k m", s=4, c=4, k=8)
            # round-robin three DMA rings per half-row so sample 0's leading
            # d-range lands as early as possible (compute pipelines behind)
            # V on sync + gpsimd SWDGE: both issuing sequencers are otherwise
            # idle (scalar-ring issue would block the ACT sequencer, a
            # critical scan engine, until the queue drains).
            rings = [nc.sync, nc.gpsimd]
            ring_i = 0
            H = S // 2
            for s in range(SPC):
                for dd in range(2):
                    for dh in range(2):
                        for dw in range(2):
                            d = dd * 4 + dh * 2 + dw
                            off = s * 3 * S + dd * 1024 + dh * 32 + dw
                            src = xcat[off:off + 3 * S].rearrange("(c m) -> c m", m=S)
                            for h in range(2):
                                rings[ring_i % len(rings)].dma_start(
                                    Vv[s, 0:3, d, h * H:(h + 1) * H],
                                    src[0:3, h * H:(h + 1) * H])
                                ring_i += 1

            V4 = V.rearrange("p (d h w) -> p d h w", h=32, w=32)
            V4p = V.rearrange("p (d dp h hp w wp) -> p dp hp wp d h w",
                              d=16, dp=2, h=16, hp=2, w=16, wp=2)

            # ---- main conv + quad scans, sample-outer for DMA overlap ----
            dve_slot = 0
            act_slot = 0
            with tc.tile_pool(name="psA", bufs=4, space="PSUM") as psA:
                for s in range(SPC):
                    for qi in _ORDER:
                        quad = _QUADS[qi]
                        qt = psA.tile([128, 1024], f32, tag="duo")
                        offs = []
                        for b, ci in enumerate(quad):
                            er, orr, d0, nd, h0, nh, w0, nw = _CH[ci]
                            N = nd * nh * nw
                            rhs = V4[32 * s:32 * s + 24,
                                     d0:d0 + nd, h0:h0 + nh, w0:w0 + nw]
                            nc.tensor.matmul(
                                qt[:, 512 * b:512 * b + N],
                                W128t[32 * s:32 * s + 24, :],
                                rhs,
                                start=True, stop=True,
                                tile_position=(32 * s, 0),
                            )
                            offs.append(N)
                        if _ASSIGN[qi] == "act":
                            # uniform N=465 interior chunks: one strided AP
                            N = offs[0]
                            ap = qt.rearrange(
                                "p (b c) -> p b c", b=2)[:, :, 0:N]
                            a = act_slot
                            act_slot += 1
                            nc.scalar.activation(SCRA[:, 0:2 * N], ap,
                                                 AFT.Square,
                                                 accum_out=ASQ[:, a:a + 1])
                            nc.scalar.activation(SCRA[:, 0:2 * N], ap,
                                                 AFT.Identity,
                                                 accum_out=ASUM[:, a:a + 1])
                        else:
                            for b, ci in enumerate(quad):
                                N = offs[b]
                                t = dve_slot
                                dve_slot += 1
                                nc.vector.bn_stats(
                                    STATS[:, 6 * t:6 * t + 6],
                                    qt[:, 512 * b:512 * b + N])

            # ---- stats finalize ----
            st3 = STATS.rearrange("p (n t) -> p n t", t=3)
            counts = st3[:, :, 0]
            means = st3[:, :, 1]
            cvs = st3[:, :, 2]
            nc.vector.tensor_tensor(out=SCR1[:, :], in0=counts, in1=means, op=ALU.mult)
            nc.vector.tensor_tensor(out=SCR2[:, :], in0=SCR1[:, :], in1=means, op=ALU.mult)
            nc.vector.tensor_tensor(out=SCR2[:, :], in0=SCR2[:, :], in1=cvs, op=ALU.add)
            nc.vector.tensor_tensor(out=SCR2[:, :], in0=SCR2[:, :], in1=REGWt[:, :], op=ALU.mult)
            nc.vector.reduce_sum(SS[:, 1:2], SCR2[:, :], axis=mybir.AxisListType.X)
            nc.vector.tensor_tensor(out=SCR1[:, :], in0=SCR1[:, :], in1=REGWt[:, :], op=ALU.mult)
            nc.vector.reduce_sum(SS[:, 0:1], SCR1[:, :], axis=mybir.AxisListType.X)
            # merge ScalarE accumulators
            nc.vector.tensor_tensor(out=SCR1[:, 0:_NACT], in0=ASUM[:, :], in1=AREGWt[:, :], op=ALU.mult)
            nc.vector.reduce_sum(SSA[:, 0:1], SCR1[:, 0:_NACT], axis=mybir.AxisListType.X)
            nc.vector.tensor_tensor(out=SCR1[:, 0:_NACT], in0=ASQ[:, :], in1=AREGWt[:, :], op=ALU.mult)
            nc.vector.reduce_sum(SSA[:, 1:2], SCR1[:, 0:_NACT], axis=mybir.AxisListType.X)
            nc.vector.tensor_tensor(out=SS[:, :], in0=SS[:, :], in1=SSA[:, :], op=ALU.add)

            # phase-sum + broadcast to per-sample channel rows
            psB_cm = tc.tile_pool(name="psB", bufs=2, space="PSUM")
            psB = psB_cm.__enter__()
            pss = psB.tile([128, 2], f32, tag="pss")
            nc.tensor.matmul(pss[:, :], ONESt[:, :], SS[:, :], start=True, stop=True)
            nc.vector.tensor_copy(ssb[:, :], pss[:, :])

            # ---- sync-BN all-reduce across cores ----
            import os
            if n_cores > 1 and not os.environ.get("KERNEL_NO_CC"):
                cin_b = dram.tile([128, 2], f32)
                cout_b = dram.tile([128, 2], f32)
                nc.gpsimd.dma_start(cin_b[:, :], ssb[:, :])
                nc.gpsimd.collective_compute(
                    "AllReduce",
                    ALU.add,
                    replica_groups=[list(range(n_cores))],
                    ins=[cin_b.opt()],
                    outs=[cout_b.opt()],
                )
                nc.gpsimd.dma_start(gss[:, :], cout_b[:, :])
            else:
                nc.vector.tensor_copy(gss[:, :], ssb[:, :])

            # ---- pooled conv after the stats matmuls (covers all-reduce) ----
            jchunks = [(0, 2), (2, 2), (4, 2), (6, 2), (8, 2), (10, 2), (12, 2), (13, 2)]
            for jce, (jd0, njd) in enumerate(jchunks):
                NP = njd * 225
                pchunk = psB.tile([128, 512], f32, tag="pchunk")
                for p in range(8):
                    od, oh, ow = p >> 2 & 1, p >> 1 & 1, p & 1
                    for s in range(SPC):
                        rhs = V4p[32 * s:32 * s + 24, 0, 0, 0,
                                  od + jd0:od + jd0 + njd,
                                  oh:oh + 15, ow:ow + 15]
                        nc.tensor.matmul(
                            pchunk[32 * s:32 * s + 16, 0:NP],
                            WPTt[32 * s:32 * s + 24, 16 * p:16 * p + 16],
                            rhs,
                            start=(p == 0), stop=(p == 7),
                            tile_position=(32 * s, 32 * s),
                        )
                # copies on DVE only: it idles during the all-reduce window
                dst = praw[:, 225 * jd0:225 * jd0 + NP]
                nc.vector.tensor_copy(dst, pchunk[:, 0:NP])

            # ---- finalize scalars (fused) ----
            nc.vector.tensor_scalar_mul(meanT[:, :], gss[:, 0:1], 1.0 / CNT)
            nc.vector.tensor_scalar_mul(ex2T[:, :], gss[:, 1:2], 1.0 / CNT)
            nc.vector.tensor_tensor(out=varT[:, :], in0=meanT[:, :], in1=meanT[:, :], op=ALU.mult)
            nc.vector.tensor_tensor(out=varT[:, :], in0=ex2T[:, :], in1=varT[:, :], op=ALU.subtract)
            nc.vector.tensor_scalar_add(varT[:, :], varT[:, :], 1e-5)
            nc.scalar.activation(sqT[:, :], varT[:, :], AFT.Sqrt)
            nc.vector.reciprocal(invT[:, :], sqT[:, :])
            # scale = inv*gamma/64 ; bias = beta - mean*inv*gamma
            nc.vector.tensor_tensor(out=sclT[:, :], in0=invT[:, :], in1=GBt[:, 2:3], op=ALU.mult)
            nc.vector.tensor_tensor(out=tmpT[:, :], in0=meanT[:, :], in1=invT[:, :], op=ALU.mult)
            nc.vector.tensor_tensor(out=tmpT[:, :], in0=tmpT[:, :], in1=GBt[:, 0:1], op=ALU.mult)
            nc.vector.tensor_tensor(out=biaT[:, :], in0=GBt[:, 1:2], in1=tmpT[:, :], op=ALU.subtract)

            # ---- normalize + out (bf16, host upcasts); split halves so the
            # first out DMAs overlap the second affine half; DMAs on the
            # otherwise-idle sync ring ----
            HJ = 1700
            for h, (c0, c1) in enumerate(((0, HJ), (HJ, 3375))):
                nc.scalar.activation(staged[:, c0:c1], praw[:, c0:c1], AFT.Identity,
                                     bias=biaT[:, 0:1], scale=sclT[:, 0:1])
                for s in range(SPC):
                    nc.sync.dma_start(outd[s][:, c0:c1],
                                      staged[32 * s:32 * s + 16, c0:c1])
            psB_cm.__exit__(None, None, None)

    nc.compile()
    _BUILD_CACHE[n_cores] = nc
    return nc


# ---------------------------------------------------------------------------
# host entry point
# ---------------------------------------------------------------------------
def make_in_maps(x, weight, gamma, beta, n_cores=NCORES):
    x = np.ascontiguousarray(np.asarray(x, np.float32))
    consts = _host_consts(weight, np.asarray(gamma, np.float32), np.asarray(beta, np.float32))
    in_maps = []
    for core in range(n_cores):
        xs = x[core * SPC:(core + 1) * SPC]
        import ml_dtypes
        xc = np.zeros(XCAT, ml_dtypes.bfloat16)
        xc[:SPC * 3 * S] = _f32_to_bf16(xs.reshape(-1))
        in_maps.append({
            "xcat": xc,
            "w128": consts["w128"],
            "wpt": consts["wpt"],
            "ones": consts["ones"],
            "regw": consts["regw"],
            "aregw": consts["aregw"],
            "gb": consts["gb"],
        })
    return in_maps


def _bf16_bits_to_f32(a):
    a = np.asarray(a)
    if a.dtype == np.uint16:
        return (a.astype(np.uint32) << 16).view(np.float32)
    return a.astype(np.float32)


def kernel(x, weight, gamma, beta):
    import sys
    if "/opt/trn_rl_repo" not in sys.path:
        sys.path.insert(0, "/opt/trn_rl_repo")
    from concourse.bass_utils import run_bass_kernel_spmd

    nc = build_nc(NCORES)
    in_maps = make_in_maps(x, weight, gamma, beta, NCORES)
    res = run_bass_kernel_spmd(nc, in_maps, core_ids=list(range(NCORES)))
    outs = [_bf16_bits_to_f32(r["out"]).reshape(SPC, 16, 15, 15, 15)
            for r in res.results]
    return np.concatenate(outs, axis=0)


if __name__ == "__main__":
    import sys
    sys.path.insert(0, "/opt/trn_rl_repo")
    sys.path.insert(0, "/root/problem")
    import reference as ref
    inputs = {k: np.asarray(v) for k, v in ref.setup_inputs().items()}
    out = kernel(**inputs)
    print("out shape", out.shape)
